# revision 7
# baseline (speedup 1.0000x reference)
"""Causal cross-attention Trainium2 kernel (8-core SPMD).

Problem: B=2, T=T_ctx=2048, C=1024, H=16 heads, D=64.
  q = x@Wq + bq;  k,v = context@Wkv + bkv
  att = softmax(causal_mask(q k^T / sqrt(D)));  out = (att v) @ Wp + bp

Sharding (data parallel on B x tensor parallel on heads):
  core c: batch b = c // 4, heads [4*(c%4) .. 4*(c%4)+3]
  Each core computes q/k/v projections for its 256 head-dim columns,
  attention for its 4 heads, and a partial out-projection (its rows of
  Wp). Host transposes x/context per batch (pure layout prep), sums the
  4 partial outputs per batch, and adds bp.

Per-core dataflow (all matmuls f32r = full-rate PE):
  qT/kT in [dc, t] layout (head dims on partitions, 2 m-tiles of 128),
  v_aug in [s, 65*h] layout (64 v cols + ones col per head -> softmax
  denominators fall out of the AV matmul), scores^T [s, t] per s-tile j
  accumulated in PSUM, exp'd on ACT (scale=1/sqrt(D)) into a resident
  per-head expT buffer (causal spans only), AV accumulates [65, 512]
  per q-block in PSUM (row 64 = denominator), normalized via DVE
  reciprocal + gpsimd partition broadcast, out-projection from yT.
"""
import sys

sys.path.insert(0, '/opt/trn_rl_repo')

import numpy as np

import concourse.bass as bass
import concourse.mybir as mybir
from concourse.tile import TileContext

F32 = mybir.dt.float32
F32R = mybir.dt.float32r
EXP = mybir.ActivationFunctionType.Exp
COPY = mybir.ActivationFunctionType.Copy

B, T, C, H, D = 2, 2048, 1024, 16, 64
HC = 4            # heads per core
DC = HC * D       # head-dim columns per core (256)
VC = HC * 65      # v_aug columns (per head: 64 v cols + ones col)
NT = T // 128     # 16 s/t tiles
NB = T // 512     # 4 q-blocks
KO = C // 128     # 8 contraction subtiles

_cached = {}


def split_sync_waits(nc, maxw=1):
    """This walrus build rejects instructions with >1 sync-wait; move the
    excess onto dedicated NOPs inserted just before, on the same engine."""
    n = 0
    for fn in nc.m.functions:
        for bb in fn.blocks:
            insts = bb.instructions
            i = 0
            while i < len(insts):
                inst = insts[i]
                si = getattr(inst, 'sync_info', None)
                if si is not None and si.on_wait and len(si.on_wait) > maxw:
                    waits = list(si.on_wait)
                    extra = waits[:-maxw]
                    while len(si.on_wait) > maxw:
                        si.on_wait.pop(0)
                    nops = []
                    for w in extra:
                        nop = mybir.InstNoOp(
                            name=f"I-{nc.next_id()}",
                            engine=inst.engine,
                            bass_nofuse=True,
                            sync_info=mybir.SyncInfo(on_wait=[w], on_update=[]),
                        )
                        nc.register_instruction(nop)
                        nops.append(nop)
                    insts[i:i] = nops
                    i += len(nops)
                    n += 1
                i += 1
    return n


# expT span per s-tile j: global t in [128*j, 2048), stored contiguously
SPAN_OFF = []
SPAN_LEN = []
_off = 0
for _j in range(NT):
    SPAN_OFF.append(_off)
    SPAN_LEN.append(T - 128 * _j)
    _off += SPAN_LEN[-1]
EXPT_COLS = _off  # 17408


def build_program():
    nc = bass.Bass()

    xT_d = nc.dram_tensor("xT", [C, T], F32R, kind="ExternalInput")
    cT_d = nc.dram_tensor("cT", [C, T], F32R, kind="ExternalInput")
    wq_d = nc.dram_tensor("wq", [C, DC], F32R, kind="ExternalInput")
    wk_d = nc.dram_tensor("wk", [C, DC], F32R, kind="ExternalInput")
    wv_d = nc.dram_tensor("wv", [C, VC], F32R, kind="ExternalInput")
    wp_d = nc.dram_tensor("wp", [DC, C], F32R, kind="ExternalInput")
    bq_d = nc.dram_tensor("bq", [1, DC], F32R, kind="ExternalInput")
    bk_d = nc.dram_tensor("bk", [1, DC], F32R, kind="ExternalInput")
    bv_d = nc.dram_tensor("bv", [1, VC], F32R, kind="ExternalInput")
    ones_d = nc.dram_tensor("onesr", [1, 512], F32R, kind="ExternalInput")
    msk_d = nc.dram_tensor("msk", [128, 128], F32R, kind="ExternalInput")
    out_d = nc.dram_tensor("out", [T, C], F32, kind="ExternalOutput")

    SCALE = 1.0 / float(np.sqrt(D))
    xT_r = xT_d.rearrange("(ko p) t -> p ko t", p=128)
    cT_r = cT_d.rearrange("(ko p) t -> p ko t", p=128)

    with TileContext(nc) as tc:
        with (
            tc.tile_pool(name="const", bufs=1) as constp,
            tc.tile_pool(name="w", bufs=1) as wpool,
            tc.tile_pool(name="act", bufs=2) as actp,
            tc.tile_pool(name="qkv", bufs=1) as qkvp,
            tc.tile_pool(name="expt", bufs=1) as exptp,
            tc.tile_pool(name="y", bufs=1) as yp,
            tc.tile_pool(name="o", bufs=2) as op_,
            tc.tile_pool(name="nrm", bufs=2) as nrmp,
            tc.tile_pool(name="psQK", bufs=1, space="PSUM") as psQK,
            tc.tile_pool(name="psAV", bufs=2, space="PSUM") as psAV,
            tc.tile_pool(name="psP", bufs=2, space="PSUM") as psP,
        ):
            # ---- constants / weights ----
            ones = constp.tile([1, 512], F32R, tag="ones")
            nc.sync.dma_start(ones[:], ones_d[:])
            msk = constp.tile([128, 128], F32R, tag="msk")
            nc.sync.dma_start(msk[:], msk_d[:])
            bq = constp.tile([1, DC], F32R, tag="bq")
            bk = constp.tile([1, DC], F32R, tag="bk")
            bv = constp.tile([1, VC], F32R, tag="bv")
            nc.sync.dma_start(bq[:], bq_d[:])
            nc.sync.dma_start(bk[:], bk_d[:])
            nc.sync.dma_start(bv[:], bv_d[:])

            wq = wpool.tile([128, KO, DC], F32R, tag="wq")
            wk = wpool.tile([128, KO, DC], F32R, tag="wk")
            wv = wpool.tile([128, KO, VC], F32R, tag="wv")
            nc.sync.dma_start(wq[:], wq_d.rearrange("(ko p) d -> p ko d", p=128))
            nc.sync.dma_start(wk[:], wk_d.rearrange("(ko p) d -> p ko d", p=128))
            nc.sync.dma_start(wv[:], wv_d.rearrange("(ko p) d -> p ko d", p=128))
            wp = wpool.tile([128, 2, C], F32R, tag="wp")
            nc.sync.dma_start(wp[:], wp_d.rearrange("(m p) c -> p m c", p=128))

            # ---- persistent activations ----
            qT = qkvp.tile([128, 2, T], F32R, tag="qT")          # [dc, m, t]
            kT = qkvp.tile([128, 2, T], F32R, tag="kT")
            vA = qkvp.tile([128, NT, HC * 65], F32R, tag="vA")   # v_aug
            yT = yp.tile([128, 2, T], F32R, tag="yT")



            def qk_proj(src_r, w_t, b_t, dst, m):
                # dst[:, m, :] [128 dc, 2048 t] = (w^T x)^T + bias, 256-wide chunks
                for tc8 in range(8):
                    a = actp.tile([128, KO, 256], F32R, tag="a_in")
                    nc.sync.dma_start(
                        a[:], src_r[:, :, 256 * tc8: 256 * (tc8 + 1)])
                    ps = psP.tile([128, 512], F32, tag="psP")
                    for k in range(KO):
                        nc.tensor.matmul(
                            ps[:, :256], w_t[:, k, 128 * m: 128 * (m + 1)],
                            a[:, k, :], start=(k == 0), stop=False)
                    nc.tensor.matmul(
                        ps[:, :256], b_t[:, 128 * m: 128 * (m + 1)],
                        ones[:, :256], start=False, stop=True)
                    nc.vector.tensor_copy(
                        dst[:, m, 256 * tc8: 256 * (tc8 + 1)], ps[:, :256])

            def v_proj():
                for jj in range(8):
                    a = actp.tile([128, KO, 256], F32R, tag="a_in")
                    nc.sync.dma_start(
                        a[:], cT_r[:, :, 256 * jj: 256 * (jj + 1)])
                    for sub in range(2):
                        j = 2 * jj + sub
                        ps = psP.tile([128, 512], F32, tag="psP")
                        for k in range(KO):
                            nc.tensor.matmul(
                                ps[:, :VC], a[:, k, 128 * sub: 128 * (sub + 1)],
                                wv[:, k, :], start=(k == 0), stop=False)
                        nc.tensor.matmul(
                            ps[:, :VC], ones[:, :128], bv[:],
                            start=False, stop=True)
                        nc.vector.tensor_copy(vA[:, j, :], ps[:, :VC])

            def attn_head(h, expT, emit_tail):
                m, lo = h // 2, 64 * (h % 2)
                kT_h = kT[lo:lo + 64, m, :]
                qT_h = qT[lo:lo + 64, m, :]
                vA_h_base = 65 * h

                def qk_exp(j):
                    slen = SPAN_LEN[j]
                    ps = psQK.tile([128, 2048], F32, tag="psQK")
                    for c0 in range(0, slen, 512):
                        n = min(512, slen - c0)
                        nc.tensor.matmul(
                            ps[:, c0: c0 + n],
                            kT_h[:, 128 * j: 128 * (j + 1)],
                            qT_h[:, 128 * j + c0: 128 * j + c0 + n],
                            start=True, stop=True)
                    nc.scalar.activation(
                        expT[:, SPAN_OFF[j]: SPAN_OFF[j] + slen],
                        ps[:, :slen], EXP, scale=SCALE)
                    # diagonal 128x128: keep s <= t
                    dg = SPAN_OFF[j]
                    nc.vector.tensor_mul(
                        expT[:, dg: dg + 128], expT[:, dg: dg + 128], msk[:])

                def av_block(qb):
                    ps = psAV.tile([128, 512], F32, tag="psAV")
                    njs = 4 * qb + 4
                    for j in range(njs):
                        lhsT = vA[:, j, vA_h_base: vA_h_base + 65]
                        if j < 4 * qb:
                            co = SPAN_OFF[j] + 512 * qb - 128 * j
                            nc.tensor.matmul(
                                ps[:65, :], lhsT, expT[:, co: co + 512],
                                start=(j == 0), stop=(j == njs - 1))
                        else:
                            lead = 128 * j - 512 * qb
                            nc.tensor.matmul(
                                ps[:65, lead:], lhsT,
                                expT[:, SPAN_OFF[j]: SPAN_OFF[j] + 512 - lead],
                                start=(j == 0), stop=(j == njs - 1))
                    rcp = nrmp.tile([1, 512], F32R, tag="rcp")
                    with nc.allow_low_precision("f32r softmax denominators"):
                        nc.vector.reciprocal(rcp[:], ps[64:65, :])
                    # broadcast 1/denom across 64 partitions via K=1 matmul
                    rbc = psP.tile([128, 512], F32, tag="psP")
                    nc.tensor.matmul(rbc[:64, :], ones[:, :64], rcp[:],
                                     start=True, stop=True)
                    rbs = nrmp.tile([64, 512], F32, tag="rbs")
                    nc.vector.tensor_copy(rbs[:], rbc[:64, :])
                    nc.vector.tensor_mul(
                        yT[lo:lo + 64, m, 512 * qb: 512 * (qb + 1)],
                        ps[:64, :], rbs[:])

                for qb in range(NB):
                    for j in range(4 * qb, 4 * qb + 4):
                        qk_exp(j)
                    av_block(qb)
                    if emit_tail:
                        for tt in range(4 * qb, 4 * qb + 4):
                            out_proj_tile(tt)

            def out_proj_tile(tt):
                ts = slice(128 * tt, 128 * (tt + 1))
                for co in range(2):
                    ps = psP.tile([128, 512], F32, tag="psP")
                    for m in range(2):
                        nc.tensor.matmul(
                            ps[:], yT[:, m, ts],
                            wp[:, m, 512 * co: 512 * (co + 1)],
                            start=(m == 0), stop=(m == 1))
                    ob = op_.tile([128, 512], F32, tag="ob")
                    if (tt + co) % 2:
                        nc.vector.tensor_copy(ob[:], ps[:])
                    else:
                        nc.scalar.activation(ob[:], ps[:], COPY)
                    nc.sync.dma_start(out_d[ts, 512 * co: 512 * (co + 1)], ob[:])

            # ---- emission order (pipelines ACT exp against PE) ----
            qk_proj(cT_r, wk, bk, kT, 0)
            qk_proj(xT_r, wq, bq, qT, 0)
            v_proj()
            expT = exptp.tile([128, EXPT_COLS], F32R, tag="expT")
            attn_head(0, expT, False)
            attn_head(1, expT, False)
            qk_proj(cT_r, wk, bk, kT, 1)
            qk_proj(xT_r, wq, bq, qT, 1)
            attn_head(2, expT, False)
            attn_head(3, expT, True)

    split_sync_waits(nc)
    return nc


def kernel(x, context, Wq, bq, Wkv, bkv, Wp, bp):
    from concourse.bass_utils import run_bass_kernel_spmd

    x = np.asarray(x, dtype=np.float32)
    context = np.asarray(context, dtype=np.float32)
    Wq = np.asarray(Wq, dtype=np.float32)
    Wkv = np.asarray(Wkv, dtype=np.float32)
    Wp_a = np.asarray(Wp, dtype=np.float32)
    bq_a = np.asarray(bq, dtype=np.float32)
    bkv_a = np.asarray(bkv, dtype=np.float32)
    bp_a = np.asarray(bp, dtype=np.float32)

    if 'nc' not in _cached:
        _cached['nc'] = build_program()
    nc = _cached['nc']

    msk = np.triu(np.ones((128, 128), dtype=np.float32))
    xT = [np.ascontiguousarray(x[b].T) for b in range(B)]
    cT = [np.ascontiguousarray(context[b].T) for b in range(B)]

    ones512 = np.ones((1, 512), dtype=np.float32)
    in_maps = []
    for c in range(8):
        b, hg = c // 4, c % 4
        hs = slice(DC * hg, DC * (hg + 1))
        vs = slice(C + DC * hg, C + DC * (hg + 1))
        wv_aug = np.zeros((C, VC), dtype=np.float32)
        bv_aug = np.zeros((1, VC), dtype=np.float32)
        for h in range(HC):
            wv_aug[:, 65 * h: 65 * h + 64] = Wkv[:, C + DC * hg + 64 * h:
                                                 C + DC * hg + 64 * (h + 1)]
            bv_aug[0, 65 * h: 65 * h + 64] = bkv_a[C + DC * hg + 64 * h:
                                                   C + DC * hg + 64 * (h + 1)]
            bv_aug[0, 65 * h + 64] = 1.0
        in_maps.append({
            "xT": xT[b], "cT": cT[b],
            "wq": np.ascontiguousarray(Wq[:, hs]),
            "wk": np.ascontiguousarray(Wkv[:, hs]),
            "wv": wv_aug,
            "wp": np.ascontiguousarray(Wp_a[hs, :]),
            "bq": np.ascontiguousarray(bq_a[hs]).reshape(1, DC),
            "bk": np.ascontiguousarray(bkv_a[hs]).reshape(1, DC),
            "bv": bv_aug,
            "msk": msk, "onesr": ones512,
        })

    res = run_bass_kernel_spmd(nc, in_maps, list(range(8)))
    out = np.zeros((B, T, C), dtype=np.float32)
    for c in range(8):
        out[c // 4] += res.results[c]["out"]
    out += bp_a[None, None, :]
    return out


# revision 14
# speedup vs baseline: 1.7073x; 1.7073x over previous
"""Causal cross-attention Trainium2 kernel (8-core SPMD).

Problem: B=2, T=T_ctx=2048, C=1024, H=16 heads, D=64.
  q = x@Wq + bq;  k,v = context@Wkv + bkv
  att = softmax(causal_mask(q k^T / sqrt(D)));  out = (att v) @ Wp + bp

Sharding (data parallel on B x tensor parallel on heads):
  core c: batch b = c // 4, heads [4*(c%4) .. 4*(c%4)+3]
  Each core computes q/k/v projections for its 256 head-dim columns,
  attention for its 4 heads, and a partial out-projection (its rows of
  Wp). Host transposes x/context per batch (pure layout prep), sums the
  4 partial outputs per batch, and adds bp.

Per-core dataflow (all matmuls f32r = full-rate PE):
  qT/kT in [dc, t] layout (head dims on partitions, 2 m-tiles of 128),
  v_aug in [s, 65*h] layout (64 v cols + ones col per head -> softmax
  denominators fall out of the AV matmul), scores^T [s, t] per s-tile j
  accumulated in PSUM, exp'd on ACT (scale=1/sqrt(D)) into a resident
  per-head expT buffer (causal spans only), AV accumulates [65, 512]
  per q-block in PSUM (row 64 = denominator), normalized via DVE
  reciprocal + gpsimd partition broadcast, out-projection from yT.
"""
import sys

sys.path.insert(0, '/opt/trn_rl_repo')

import numpy as np

import concourse.bass as bass
import concourse.mybir as mybir
from concourse.tile import TileContext

F32 = mybir.dt.float32
F32R = mybir.dt.float32r
EXP = mybir.ActivationFunctionType.Exp
COPY = mybir.ActivationFunctionType.Copy

B, T, C, H, D = 2, 2048, 1024, 16, 64
HC = 4            # heads per core
DC = HC * D       # head-dim columns per core (256)
VC = HC * 65      # v_aug columns (per head: 64 v cols + ones col)
NT = T // 128     # 16 s/t tiles
NB = T // 512     # 4 q-blocks
KO = C // 128     # 8 contraction subtiles

_cached = {}


def split_sync_waits(nc, maxw=1):
    """This walrus build rejects instructions with >1 sync-wait; move the
    excess onto dedicated NOPs inserted just before, on the same engine."""
    n = 0
    for fn in nc.m.functions:
        for bb in fn.blocks:
            insts = bb.instructions
            i = 0
            while i < len(insts):
                inst = insts[i]
                si = getattr(inst, 'sync_info', None)
                if si is not None and si.on_wait and len(si.on_wait) > maxw:
                    waits = list(si.on_wait)
                    extra = waits[:-maxw]
                    while len(si.on_wait) > maxw:
                        si.on_wait.pop(0)
                    nops = []
                    for w in extra:
                        nop = mybir.InstNoOp(
                            name=f"I-{nc.next_id()}",
                            engine=inst.engine,
                            bass_nofuse=True,
                            sync_info=mybir.SyncInfo(on_wait=[w], on_update=[]),
                        )
                        nc.register_instruction(nop)
                        nops.append(nop)
                    insts[i:i] = nops
                    i += len(nops)
                    n += 1
                i += 1
    return n


# Attention runs in two t-phases: phase 0 covers t in [0, 1024) (s-tiles
# j<8), phase 1 covers t in [1024, 2048) (all 16 s-tiles). Each phase
# stores, per s-tile j, the causal span [max(128j, TLO), THI) contiguously.
PH_TLO = (0, 1024)
PH_THI = (1024, 2048)
PH_JS = (8, 16)
PH_OFF = []
PH_COLS = []
for _p in range(2):
    offs = []
    off = 0
    for _j in range(PH_JS[_p]):
        offs.append(off)
        off += PH_THI[_p] - max(128 * _j, PH_TLO[_p])
    PH_OFF.append(offs)
    PH_COLS.append(off)  # 4608 / 12800


def build_program():
    nc = bass.Bass()

    xT_d = nc.dram_tensor("xT", [C, T], F32R, kind="ExternalInput")
    cT_d = nc.dram_tensor("cT", [C, T], F32R, kind="ExternalInput")
    wq_d = nc.dram_tensor("wq", [C, DC], F32R, kind="ExternalInput")
    wk_d = nc.dram_tensor("wk", [C, DC], F32R, kind="ExternalInput")
    wv_d = nc.dram_tensor("wv", [C, VC], F32R, kind="ExternalInput")
    wp_d = nc.dram_tensor("wp", [DC, C], F32R, kind="ExternalInput")
    bq_d = nc.dram_tensor("bq", [1, DC], F32R, kind="ExternalInput")
    bk_d = nc.dram_tensor("bk", [1, DC], F32R, kind="ExternalInput")
    bv_d = nc.dram_tensor("bv", [1, VC], F32R, kind="ExternalInput")
    ones_d = nc.dram_tensor("onesr", [1, 512], F32R, kind="ExternalInput")
    msk_d = nc.dram_tensor("msk", [128, 128], F32R, kind="ExternalInput")
    out_d = nc.dram_tensor("out", [T, C], F32, kind="ExternalOutput")

    SCALE = 1.0 / float(np.sqrt(D))
    xT_r = xT_d.rearrange("(ko p) t -> p ko t", p=128)
    cT_r = cT_d.rearrange("(ko p) t -> p ko t", p=128)

    with TileContext(nc) as tc:
        with (
            tc.tile_pool(name="const", bufs=1) as constp,
            tc.tile_pool(name="w", bufs=1) as wpool,
            tc.tile_pool(name="act", bufs=3) as actp,
            tc.tile_pool(name="qkv", bufs=1) as qkvp,

            tc.tile_pool(name="y", bufs=1) as yp,
            tc.tile_pool(name="nrm", bufs=2) as nrmp,
            tc.tile_pool(name="psQK", bufs=2, space="PSUM") as psQK,
            tc.tile_pool(name="psAV", bufs=2, space="PSUM") as psAV,
            tc.tile_pool(name="psP", bufs=2, space="PSUM") as psP,
        ):
            # ---- constants / first-needed weights ----
            ones = constp.tile([1, 512], F32R, tag="ones")
            nc.sync.dma_start(ones[:], ones_d[:])
            bk = constp.tile([1, DC], F32R, tag="bk")
            bv = constp.tile([1, VC], F32R, tag="bv")
            nc.sync.dma_start(bk[:], bk_d[:])
            wk = wpool.tile([128, KO, DC], F32R, tag="wk")
            wv = wpool.tile([128, KO, VC], F32R, tag="wv")
            nc.sync.dma_start(wk[:], wk_d.rearrange("(ko p) d -> p ko d", p=128))
            # loaded later, between projection passes (hides under compute)
            msk = constp.tile([128, 128], F32R, tag="msk")
            bq = constp.tile([1, DC], F32R, tag="bq")
            wq = wpool.tile([128, KO, DC], F32R, tag="wq")
            wp = wpool.tile([128, 2, C], F32R, tag="wp")

            # ---- persistent activations ----
            qT = qkvp.tile([128, 2, T], F32R, tag="qT")          # [dc, m, t]
            kT = qkvp.tile([128, 2, T], F32R, tag="kT")
            vA = qkvp.tile([128, NT, HC * 65], F32R, tag="vA")   # v_aug
            yT = yp.tile([128, 2, T], F32R, tag="yT")



            def load_chunk(src_r, tc4):
                # one 512-wide t-chunk of the [c, t] source, as 2 half-ko tiles
                aA = actp.tile([128, 4, 512], F32R, tag="a_in")
                aB = actp.tile([128, 4, 512], F32R, tag="a_in")
                cs = slice(512 * tc4, 512 * (tc4 + 1))
                nc.sync.dma_start(aA[:], src_r[:, 0:4, cs])
                nc.sync.dma_start(aB[:], src_r[:, 4:8, cs])

                def kslice(k, sub=None):
                    t_ = aA if k < 4 else aB
                    if sub is None:
                        return t_[:, k % 4, :]
                    return t_[:, k % 4, 128 * sub: 128 * (sub + 1)]
                return kslice

            def qk_chunk(kslice, w_t, b_t, dst, tc4):
                # dst[:, m, chunk] [128 dc, 512 t] = (w^T x)^T + bias
                for m in range(2):
                    ps = psP.tile([128, 512], F32, tag="psP")
                    for k in range(KO):
                        nc.tensor.matmul(
                            ps[:], w_t[:, k, 128 * m: 128 * (m + 1)],
                            kslice(k), start=(k == 0), stop=False)
                    nc.tensor.matmul(
                        ps[:], b_t[:, 128 * m: 128 * (m + 1)],
                        ones[:], start=False, stop=True)
                    nc.vector.tensor_copy(
                        dst[:, m, 512 * tc4: 512 * (tc4 + 1)], ps[:])

            def v_chunk(kslice, tc4):
                for sub in range(4):
                    j = 4 * tc4 + sub
                    ps = psP.tile([128, 512], F32, tag="psP")
                    for k in range(KO):
                        nc.tensor.matmul(
                            ps[:, :VC], kslice(k, sub),
                            wv[:, k, :], start=(k == 0), stop=False)
                    nc.tensor.matmul(
                        ps[:, :VC], ones[:, :128], bv[:],
                        start=False, stop=True)
                    nc.vector.tensor_copy(vA[:, j, :], ps[:, :VC])

            def attn_phase(h, expT, ph, after_qb=None):
                m, lo = h // 2, 64 * (h % 2)
                kT_h = kT[lo:lo + 64, m, :]
                qT_h = qT[lo:lo + 64, m, :]
                vA_h_base = 65 * h
                TLO, THI = PH_TLO[ph], PH_THI[ph]
                OFF = PH_OFF[ph]

                for j in range(PH_JS[ph]):
                    s0 = max(128 * j, TLO)
                    slen = THI - s0
                    ps = psQK.tile([128, 1024], F32, tag="psQK")
                    for c0 in range(0, slen, 512):
                        n = min(512, slen - c0)
                        nc.tensor.matmul(
                            ps[:, c0: c0 + n],
                            kT_h[:, 128 * j: 128 * (j + 1)],
                            qT_h[:, s0 + c0: s0 + c0 + n],
                            start=True, stop=True)
                    nc.scalar.activation(
                        expT[:, OFF[j]: OFF[j] + slen],
                        ps[:, :slen], EXP, scale=SCALE)
                    if s0 == 128 * j:
                        # diagonal 128x128 at span start: keep s <= t
                        nc.vector.tensor_mul(
                            expT[:, OFF[j]: OFF[j] + 128],
                            expT[:, OFF[j]: OFF[j] + 128], msk[:])

                for qb in (2 * ph, 2 * ph + 1):
                    ps = psAV.tile([128, 512], F32, tag="psAV")
                    njs = 4 * qb + 4
                    for j in range(njs):
                        lhsT = vA[:, j, vA_h_base: vA_h_base + 65]
                        st = (j == 0)
                        sp = (j == njs - 1)
                        if j // 4 == qb:
                            lead = 128 * j - 512 * qb
                            nc.tensor.matmul(
                                ps[:65, lead:], lhsT,
                                expT[:, OFF[j]: OFF[j] + 512 - lead],
                                start=st, stop=sp)
                        else:
                            co = OFF[j] + 512 * qb - max(128 * j, TLO)
                            nc.tensor.matmul(
                                ps[:65, :], lhsT, expT[:, co: co + 512],
                                start=st, stop=sp)
                    rcp = nrmp.tile([1, 512], F32R, tag="rcp")
                    with nc.allow_low_precision("f32r softmax denominators"):
                        nc.vector.reciprocal(rcp[:], ps[64:65, :])
                    # broadcast 1/denom across 64 partitions via K=1 matmul
                    rbc = psP.tile([128, 512], F32, tag="psP")
                    nc.tensor.matmul(rbc[:64, :], ones[:, :64], rcp[:],
                                     start=True, stop=True)
                    rbs = nrmp.tile([64, 512], F32, tag="rbs")
                    nc.vector.tensor_copy(rbs[:], rbc[:64, :])
                    nc.vector.tensor_mul(
                        yT[lo:lo + 64, m, 512 * qb: 512 * (qb + 1)],
                        ps[:64, :], rbs[:])
                    if after_qb is not None and qb in after_qb:
                        after_qb[qb]()

            def out_proj_tile(tt):
                ts = slice(128 * tt, 128 * (tt + 1))
                for co in range(2):
                    ps = psP.tile([128, 512], F32, tag="psP")
                    for m in range(2):
                        nc.tensor.matmul(
                            ps[:], yT[:, m, ts],
                            wp[:, m, 512 * co: 512 * (co + 1)],
                            start=(m == 0), stop=(m == 1))
                    ob = obp[0].tile([128, 512], F32, tag="ob")
                    if (tt + co) % 2:
                        nc.vector.tensor_copy(ob[:], ps[:])
                    else:
                        nc.scalar.activation(ob[:], ps[:], COPY)
                    nc.sync.dma_start(out_d[ts, 512 * co: 512 * (co + 1)], ob[:])

            # ---- emission order (pipelines ACT exp against PE) ----
            def proj_pair(tc4):
                ksc = load_chunk(cT_r, tc4)
                if tc4 == 0:
                    nc.sync.dma_start(
                        wv[:], wv_d.rearrange("(ko p) d -> p ko d", p=128))
                    nc.sync.dma_start(bv[:], bv_d[:])
                ksx = load_chunk(xT_r, tc4)
                if tc4 == 0:
                    nc.sync.dma_start(
                        wq[:], wq_d.rearrange("(ko p) d -> p ko d", p=128))
                    nc.sync.dma_start(bq[:], bq_d[:])
                    nc.sync.dma_start(msk[:], msk_d[:])
                qk_chunk(ksc, wk, bk, kT, tc4)
                v_chunk(ksc, tc4)
                qk_chunk(ksx, wq, bq, qT, tc4)

            obp = [None]
            with tc.tile_pool(name="expt2", bufs=1) as exptp2:
                expT2 = exptp2.tile([128, PH_COLS[1]], F32R, tag="expT2")
                with tc.tile_pool(name="expt1", bufs=1) as exptp1:
                    expT1 = exptp1.tile([128, PH_COLS[0]], F32R, tag="expT1")
                    proj_pair(0)
                    proj_pair(1)
                    attn_phase(0, expT1, 0)
                    proj_pair(2)
                    attn_phase(1, expT1, 0)
                    nc.sync.dma_start(
                        wp[:], wp_d.rearrange("(m p) c -> p m c", p=128))
                    attn_phase(2, expT1, 0)
                    proj_pair(3)
                    attn_phase(3, expT1, 0)

                with tc.tile_pool(name="ob", bufs=8) as _obp:
                    obp[0] = _obp
                    attn_phase(0, expT2, 1)
                    for tt in range(0, 4):
                        out_proj_tile(tt)
                    attn_phase(1, expT2, 1)
                    for tt in range(4, 8):
                        out_proj_tile(tt)
                    attn_phase(2, expT2, 1)
                    attn_phase(3, expT2, 1, after_qb={
                        2: lambda: [out_proj_tile(tt) for tt in range(8, 12)],
                        3: lambda: [out_proj_tile(tt) for tt in range(12, 16)],
                    })

    split_sync_waits(nc)
    return nc


def kernel(x, context, Wq, bq, Wkv, bkv, Wp, bp):
    from concourse.bass_utils import run_bass_kernel_spmd

    x = np.asarray(x, dtype=np.float32)
    context = np.asarray(context, dtype=np.float32)
    Wq = np.asarray(Wq, dtype=np.float32)
    Wkv = np.asarray(Wkv, dtype=np.float32)
    Wp_a = np.asarray(Wp, dtype=np.float32)
    bq_a = np.asarray(bq, dtype=np.float32)
    bkv_a = np.asarray(bkv, dtype=np.float32)
    bp_a = np.asarray(bp, dtype=np.float32)

    if 'nc' not in _cached:
        _cached['nc'] = build_program()
    nc = _cached['nc']

    msk = np.triu(np.ones((128, 128), dtype=np.float32))
    xT = [np.ascontiguousarray(x[b].T) for b in range(B)]
    cT = [np.ascontiguousarray(context[b].T) for b in range(B)]

    ones512 = np.ones((1, 512), dtype=np.float32)
    in_maps = []
    for c in range(8):
        b, hg = c // 4, c % 4
        hs = slice(DC * hg, DC * (hg + 1))
        vs = slice(C + DC * hg, C + DC * (hg + 1))
        wv_aug = np.zeros((C, VC), dtype=np.float32)
        bv_aug = np.zeros((1, VC), dtype=np.float32)
        for h in range(HC):
            wv_aug[:, 65 * h: 65 * h + 64] = Wkv[:, C + DC * hg + 64 * h:
                                                 C + DC * hg + 64 * (h + 1)]
            bv_aug[0, 65 * h: 65 * h + 64] = bkv_a[C + DC * hg + 64 * h:
                                                   C + DC * hg + 64 * (h + 1)]
            bv_aug[0, 65 * h + 64] = 1.0
        in_maps.append({
            "xT": xT[b], "cT": cT[b],
            "wq": np.ascontiguousarray(Wq[:, hs]),
            "wk": np.ascontiguousarray(Wkv[:, hs]),
            "wv": wv_aug,
            "wp": np.ascontiguousarray(Wp_a[hs, :]),
            "bq": np.ascontiguousarray(bq_a[hs]).reshape(1, DC),
            "bk": np.ascontiguousarray(bkv_a[hs]).reshape(1, DC),
            "bv": bv_aug,
            "msk": msk, "onesr": ones512,
        })

    res = run_bass_kernel_spmd(nc, in_maps, list(range(8)))
    out = np.zeros((B, T, C), dtype=np.float32)
    for c in range(8):
        out[c // 4] += res.results[c]["out"]
    out += bp_a[None, None, :]
    return out


# revision 20
# speedup vs baseline: 1.7277x; 1.0119x over previous
"""Causal cross-attention Trainium2 kernel (8-core SPMD).

Problem: B=2, T=T_ctx=2048, C=1024, H=16 heads, D=64.
  q = x@Wq + bq;  k,v = context@Wkv + bkv
  att = softmax(causal_mask(q k^T / sqrt(D)));  out = (att v) @ Wp + bp

Sharding (data parallel on B x tensor parallel on heads):
  core c: batch b = c // 4, heads [4*(c%4) .. 4*(c%4)+3]
  Each core computes q/k/v projections for its 256 head-dim columns,
  attention for its 4 heads, and a partial out-projection (its rows of
  Wp). Host transposes x/context per batch (pure layout prep), sums the
  4 partial outputs per batch, and adds bp.

Per-core dataflow (all matmuls f32r = full-rate PE):
  qT/kT in [dc, t] layout (head dims on partitions, 2 m-tiles of 128),
  v_aug in [s, 65*h] layout (64 v cols + ones col per head -> softmax
  denominators fall out of the AV matmul), scores^T [s, t] per s-tile j
  accumulated in PSUM, exp'd on ACT (scale=1/sqrt(D)) into a resident
  per-head expT buffer (causal spans only), AV accumulates [65, 512]
  per q-block in PSUM (row 64 = denominator), normalized via DVE
  reciprocal + a K=1 PE broadcast matmul, out-projection from yT.
  Attention is split into two t-phases so exp work overlaps the
  projection phase and the out-projection overlaps phase-2 exps.
"""
import sys

sys.path.insert(0, '/opt/trn_rl_repo')

import numpy as np

import concourse.bass as bass
import concourse.mybir as mybir
from concourse.tile import TileContext

F32 = mybir.dt.float32
F32R = mybir.dt.float32r
EXP = mybir.ActivationFunctionType.Exp
COPY = mybir.ActivationFunctionType.Copy

B, T, C, H, D = 2, 2048, 1024, 16, 64
HC = 4            # heads per core
DC = HC * D       # head-dim columns per core (256)
VC = HC * 65      # v_aug columns (per head: 64 v cols + ones col)
NT = T // 128     # 16 s/t tiles
NB = T // 512     # 4 q-blocks
KO = C // 128     # 8 contraction subtiles

_cached = {}


def split_sync_waits(nc, maxw=1):
    """This walrus build rejects instructions with >1 sync-wait; move the
    excess onto dedicated NOPs inserted just before, on the same engine."""
    n = 0
    for fn in nc.m.functions:
        for bb in fn.blocks:
            insts = bb.instructions
            i = 0
            while i < len(insts):
                inst = insts[i]
                si = getattr(inst, 'sync_info', None)
                if si is not None and si.on_wait and len(si.on_wait) > maxw:
                    waits = list(si.on_wait)
                    extra = waits[:-maxw]
                    while len(si.on_wait) > maxw:
                        si.on_wait.pop(0)
                    nops = []
                    for w in extra:
                        nop = mybir.InstNoOp(
                            name=f"I-{nc.next_id()}",
                            engine=inst.engine,
                            bass_nofuse=True,
                            sync_info=mybir.SyncInfo(on_wait=[w], on_update=[]),
                        )
                        nc.register_instruction(nop)
                        nops.append(nop)
                    insts[i:i] = nops
                    i += len(nops)
                    n += 1
                i += 1
    return n


# Attention runs in two t-phases: phase 0 covers t in [0, 1024) (s-tiles
# j<8), phase 1 covers t in [1024, 2048) (all 16 s-tiles). Each phase
# stores, per s-tile j, the causal span [max(128j, TLO), THI) contiguously.
PH_TLO = (0, 1024)
PH_THI = (1024, 2048)
PH_JS = (8, 16)
PH_OFF = []
PH_COLS = []
for _p in range(2):
    offs = []
    off = 0
    for _j in range(PH_JS[_p]):
        offs.append(off)
        off += PH_THI[_p] - max(128 * _j, PH_TLO[_p])
    PH_OFF.append(offs)
    PH_COLS.append(off)  # 4608 / 12800


def build_program():
    nc = bass.Bass()

    xT_d = nc.dram_tensor("xT", [C, T], F32R, kind="ExternalInput")
    cT_d = nc.dram_tensor("cT", [C, T], F32R, kind="ExternalInput")
    wq_d = nc.dram_tensor("wq", [C, DC], F32R, kind="ExternalInput")
    wk_d = nc.dram_tensor("wk", [C, DC], F32R, kind="ExternalInput")
    wv_d = nc.dram_tensor("wv", [C, VC], F32R, kind="ExternalInput")
    wp_d = nc.dram_tensor("wp", [DC, C], F32R, kind="ExternalInput")
    bq_d = nc.dram_tensor("bq", [1, DC], F32R, kind="ExternalInput")
    bk_d = nc.dram_tensor("bk", [1, DC], F32R, kind="ExternalInput")
    bv_d = nc.dram_tensor("bv", [1, VC], F32R, kind="ExternalInput")
    ones_d = nc.dram_tensor("onesr", [1, 512], F32R, kind="ExternalInput")
    msk_d = nc.dram_tensor("msk", [128, 128], F32R, kind="ExternalInput")
    out_d = nc.dram_tensor("out", [T, C], F32, kind="ExternalOutput")

    SCALE = 1.0 / float(np.sqrt(D))
    xT_r = xT_d.rearrange("(ko p) t -> p ko t", p=128)
    cT_r = cT_d.rearrange("(ko p) t -> p ko t", p=128)

    with TileContext(nc) as tc:
        with (
            tc.tile_pool(name="const", bufs=1) as constp,
            tc.tile_pool(name="w", bufs=1) as wpool,
            tc.tile_pool(name="act", bufs=3) as actp,
            tc.tile_pool(name="qkv", bufs=1) as qkvp,

            tc.tile_pool(name="y", bufs=1) as yp,
            tc.tile_pool(name="nrm", bufs=2) as nrmp,
            tc.tile_pool(name="psQK", bufs=2, space="PSUM") as psQK,
            tc.tile_pool(name="psAV", bufs=2, space="PSUM") as psAV,
            tc.tile_pool(name="psP", bufs=2, space="PSUM") as psP,
        ):
            # ---- constants / first-needed weights ----
            ones = constp.tile([1, 512], F32R, tag="ones")
            nc.sync.dma_start(ones[:], ones_d[:])
            bk = constp.tile([1, DC], F32R, tag="bk")
            bv = constp.tile([1, VC], F32R, tag="bv")
            nc.sync.dma_start(bk[:], bk_d[:])
            wk = wpool.tile([128, KO, DC], F32R, tag="wk")
            wv = wpool.tile([128, KO, VC], F32R, tag="wv")
            wk_r = wk_d.rearrange("(ko p) d -> p ko d", p=128)
            nc.sync.dma_start(wk[:, :, 0:128], wk_r[:, :, 0:128])
            # loaded later, between projection passes (hides under compute)
            msk = constp.tile([128, 128], F32R, tag="msk")
            bq = constp.tile([1, DC], F32R, tag="bq")
            wq = wpool.tile([128, KO, DC], F32R, tag="wq")
            wp = wpool.tile([128, 2, C], F32R, tag="wp")

            # ---- persistent activations ----
            qT = qkvp.tile([128, 2, T], F32R, tag="qT")          # [dc, m, t]
            kT = qkvp.tile([128, 2, T], F32R, tag="kT")
            vA = qkvp.tile([128, NT, HC * 65], F32R, tag="vA")   # v_aug
            yT = yp.tile([128, 2, T], F32R, tag="yT")



            def load_chunk(src_r, tc4):
                # one 512-wide t-chunk of the [c, t] source, as 2 half-ko tiles
                aA = actp.tile([128, 4, 512], F32R, tag="a_in")
                aB = actp.tile([128, 4, 512], F32R, tag="a_in")
                cs = slice(512 * tc4, 512 * (tc4 + 1))
                nc.sync.dma_start(aA[:], src_r[:, 0:4, cs])
                nc.sync.dma_start(aB[:], src_r[:, 4:8, cs])

                def kslice(k, sub=None):
                    t_ = aA if k < 4 else aB
                    if sub is None:
                        return t_[:, k % 4, :]
                    return t_[:, k % 4, 128 * sub: 128 * (sub + 1)]
                return kslice

            def qk_chunk(kslice, w_t, b_t, dst, tc4):
                # dst[:, m, chunk] [128 dc, 512 t] = (w^T x)^T + bias
                for m in range(2):
                    ps = psP.tile([128, 512], F32, tag="psP")
                    for k in range(KO):
                        nc.tensor.matmul(
                            ps[:], w_t[:, k, 128 * m: 128 * (m + 1)],
                            kslice(k), start=(k == 0), stop=False)
                    nc.tensor.matmul(
                        ps[:], b_t[:, 128 * m: 128 * (m + 1)],
                        ones[:], start=False, stop=True)
                    nc.vector.tensor_copy(
                        dst[:, m, 512 * tc4: 512 * (tc4 + 1)], ps[:])

            def v_chunk(kslice, tc4):
                for sub in range(4):
                    j = 4 * tc4 + sub
                    ps = psP.tile([128, 512], F32, tag="psP")
                    for k in range(KO):
                        nc.tensor.matmul(
                            ps[:, :VC], kslice(k, sub),
                            wv[:, k, :], start=(k == 0), stop=False)
                    nc.tensor.matmul(
                        ps[:, :VC], ones[:, :128], bv[:],
                        start=False, stop=True)
                    nc.vector.tensor_copy(vA[:, j, :], ps[:, :VC])

            def attn_phase(h, expT, ph, after_qb=None, js=None, qbs=None):
                m, lo = h // 2, 64 * (h % 2)
                kT_h = kT[lo:lo + 64, m, :]
                qT_h = qT[lo:lo + 64, m, :]
                vA_h_base = 65 * h
                TLO, THI = PH_TLO[ph], PH_THI[ph]
                OFF = PH_OFF[ph]

                for j in (js if js is not None else range(PH_JS[ph])):
                    s0 = max(128 * j, TLO)
                    slen = THI - s0
                    ps = psQK.tile([128, 1024], F32, tag="psQK")
                    for c0 in range(0, slen, 512):
                        n = min(512, slen - c0)
                        nc.tensor.matmul(
                            ps[:, c0: c0 + n],
                            kT_h[:, 128 * j: 128 * (j + 1)],
                            qT_h[:, s0 + c0: s0 + c0 + n],
                            start=True, stop=True)
                    nc.scalar.activation(
                        expT[:, OFF[j]: OFF[j] + slen],
                        ps[:, :slen], EXP, scale=SCALE)
                    if s0 == 128 * j:
                        # diagonal 128x128 at span start: keep s <= t
                        nc.vector.tensor_mul(
                            expT[:, OFF[j]: OFF[j] + 128],
                            expT[:, OFF[j]: OFF[j] + 128], msk[:])

                for qb in (qbs if qbs is not None else (2 * ph, 2 * ph + 1)):
                    ps = psAV.tile([128, 512], F32, tag="psAV")
                    njs = 4 * qb + 4
                    for j in range(njs):
                        lhsT = vA[:, j, vA_h_base: vA_h_base + 65]
                        st = (j == 0)
                        sp = (j == njs - 1)
                        if j // 4 == qb:
                            lead = 128 * j - 512 * qb
                            nc.tensor.matmul(
                                ps[:65, lead:], lhsT,
                                expT[:, OFF[j]: OFF[j] + 512 - lead],
                                start=st, stop=sp)
                        else:
                            co = OFF[j] + 512 * qb - max(128 * j, TLO)
                            nc.tensor.matmul(
                                ps[:65, :], lhsT, expT[:, co: co + 512],
                                start=st, stop=sp)
                    rcp = nrmp.tile([1, 512], F32R, tag="rcp")
                    with nc.allow_low_precision("f32r softmax denominators"):
                        nc.vector.reciprocal(rcp[:], ps[64:65, :])
                    # broadcast 1/denom across 64 partitions via K=1 matmul
                    rbc = psP.tile([128, 512], F32, tag="psP")
                    nc.tensor.matmul(rbc[:64, :], ones[:, :64], rcp[:],
                                     start=True, stop=True)
                    rbs = nrmp.tile([64, 512], F32, tag="rbs")
                    nc.vector.tensor_copy(rbs[:], rbc[:64, :])
                    nc.vector.tensor_mul(
                        yT[lo:lo + 64, m, 512 * qb: 512 * (qb + 1)],
                        ps[:64, :], rbs[:])
                    if after_qb is not None and qb in after_qb:
                        after_qb[qb]()

            def out_proj_tile(tt):
                ts = slice(128 * tt, 128 * (tt + 1))
                for co in range(2):
                    ps = psP.tile([128, 512], F32, tag="psP")
                    for m in range(2):
                        nc.tensor.matmul(
                            ps[:], yT[:, m, ts],
                            wp[:, m, 512 * co: 512 * (co + 1)],
                            start=(m == 0), stop=(m == 1))
                    ob = obp[0].tile([128, 512], F32, tag="ob")
                    if (tt + co) % 2:
                        nc.vector.tensor_copy(ob[:], ps[:])
                    else:
                        nc.scalar.activation(ob[:], ps[:], COPY)
                    nc.sync.dma_start(out_d[ts, 512 * co: 512 * (co + 1)], ob[:])

            # ---- emission order (pipelines ACT exp against PE) ----
            def proj_pair(tc4, first=False):
                ksc = load_chunk(cT_r, tc4)
                if first:
                    nc.sync.dma_start(wk[:, :, 128:256], wk_r[:, :, 128:256])
                    nc.sync.dma_start(
                        wv[:], wv_d.rearrange("(ko p) d -> p ko d", p=128))
                    nc.sync.dma_start(bv[:], bv_d[:])
                ksx = load_chunk(xT_r, tc4)
                if first:
                    nc.sync.dma_start(
                        wq[:], wq_d.rearrange("(ko p) d -> p ko d", p=128))
                    nc.sync.dma_start(bq[:], bq_d[:])
                    nc.sync.dma_start(msk[:], msk_d[:])
                qk_chunk(ksc, wk, bk, kT, tc4)
                v_chunk(ksc, tc4)
                qk_chunk(ksx, wq, bq, qT, tc4)

            obp = [None]
            with tc.tile_pool(name="expt2", bufs=1) as exptp2:
                expT2 = exptp2.tile([128, PH_COLS[1]], F32R, tag="expT2")
                with tc.tile_pool(name="expt1", bufs=1) as exptp1:
                    expT1 = exptp1.tile([128, PH_COLS[0]], F32R, tag="expT1")
                    proj_pair(0, first=True)
                    proj_pair(1)
                    attn_phase(0, expT1, 0)
                    proj_pair(2)
                    attn_phase(1, expT1, 0)
                    nc.sync.dma_start(
                        wp[:], wp_d.rearrange("(m p) c -> p m c", p=128))
                    attn_phase(2, expT1, 0)
                    proj_pair(3)
                    attn_phase(3, expT1, 0)

                with tc.tile_pool(name="ob", bufs=8) as _obp:
                    obp[0] = _obp
                    attn_phase(0, expT2, 1)
                    for tt in range(0, 4):
                        out_proj_tile(tt)
                    attn_phase(1, expT2, 1)
                    for tt in range(4, 8):
                        out_proj_tile(tt)
                    attn_phase(2, expT2, 1)
                    attn_phase(3, expT2, 1, after_qb={
                        2: lambda: [out_proj_tile(tt) for tt in range(8, 12)],
                        3: lambda: [out_proj_tile(tt) for tt in range(12, 16)],
                    })

    split_sync_waits(nc)
    return nc


def kernel(x, context, Wq, bq, Wkv, bkv, Wp, bp):
    from concourse.bass_utils import run_bass_kernel_spmd

    x = np.asarray(x, dtype=np.float32)
    context = np.asarray(context, dtype=np.float32)
    Wq = np.asarray(Wq, dtype=np.float32)
    Wkv = np.asarray(Wkv, dtype=np.float32)
    Wp_a = np.asarray(Wp, dtype=np.float32)
    bq_a = np.asarray(bq, dtype=np.float32)
    bkv_a = np.asarray(bkv, dtype=np.float32)
    bp_a = np.asarray(bp, dtype=np.float32)

    if 'nc' not in _cached:
        _cached['nc'] = build_program()
    nc = _cached['nc']

    msk = np.triu(np.ones((128, 128), dtype=np.float32))
    xT = [np.ascontiguousarray(x[b].T) for b in range(B)]
    cT = [np.ascontiguousarray(context[b].T) for b in range(B)]

    ones512 = np.ones((1, 512), dtype=np.float32)
    in_maps = []
    for c in range(8):
        b, hg = c // 4, c % 4
        hs = slice(DC * hg, DC * (hg + 1))
        vs = slice(C + DC * hg, C + DC * (hg + 1))
        wv_aug = np.zeros((C, VC), dtype=np.float32)
        bv_aug = np.zeros((1, VC), dtype=np.float32)
        for h in range(HC):
            wv_aug[:, 65 * h: 65 * h + 64] = Wkv[:, C + DC * hg + 64 * h:
                                                 C + DC * hg + 64 * (h + 1)]
            bv_aug[0, 65 * h: 65 * h + 64] = bkv_a[C + DC * hg + 64 * h:
                                                   C + DC * hg + 64 * (h + 1)]
            bv_aug[0, 65 * h + 64] = 1.0
        in_maps.append({
            "xT": xT[b], "cT": cT[b],
            "wq": np.ascontiguousarray(Wq[:, hs]),
            "wk": np.ascontiguousarray(Wkv[:, hs]),
            "wv": wv_aug,
            "wp": np.ascontiguousarray(Wp_a[hs, :]),
            "bq": np.ascontiguousarray(bq_a[hs]).reshape(1, DC),
            "bk": np.ascontiguousarray(bkv_a[hs]).reshape(1, DC),
            "bv": bv_aug,
            "msk": msk, "onesr": ones512,
        })

    res = run_bass_kernel_spmd(nc, in_maps, list(range(8)))
    out = np.zeros((B, T, C), dtype=np.float32)
    for c in range(8):
        out[c // 4] += res.results[c]["out"]
    out += bp_a[None, None, :]
    return out


# revision 22
# speedup vs baseline: 1.7631x; 1.0205x over previous
"""Causal cross-attention Trainium2 kernel (8-core SPMD).

Problem: B=2, T=T_ctx=2048, C=1024, H=16 heads, D=64.
  q = x@Wq + bq;  k,v = context@Wkv + bkv
  att = softmax(causal_mask(q k^T / sqrt(D)));  out = (att v) @ Wp + bp

Sharding (data parallel on B x tensor parallel on heads):
  core c: batch b = c // 4, heads [4*(c%4) .. 4*(c%4)+3]
  Each core computes q/k/v projections for its 256 head-dim columns,
  attention for its 4 heads, and a partial out-projection (its rows of
  Wp). Host transposes x/context per batch (pure layout prep), sums the
  4 partial outputs per batch, and adds bp.

Per-core dataflow (all matmuls f32r = full-rate PE):
  qT/kT in [dc, t] layout (head dims on partitions, 2 m-tiles of 128),
  v_aug in [s, 65*h] layout (64 v cols + ones col per head -> softmax
  denominators fall out of the AV matmul), scores^T [s, t] per s-tile j
  accumulated in PSUM, exp'd on ACT (scale=1/sqrt(D)) into a resident
  per-head expT buffer (causal spans only), AV accumulates [65, 512]
  per q-block in PSUM (row 64 = denominator), normalized via DVE
  reciprocal + a K=1 PE broadcast matmul, out-projection from yT.
  Attention is split into two t-phases so exp work overlaps the
  projection phase and the out-projection overlaps phase-2 exps.
"""
import sys

sys.path.insert(0, '/opt/trn_rl_repo')

import numpy as np

import concourse.bass as bass
import concourse.mybir as mybir
from concourse.tile import TileContext

F32 = mybir.dt.float32
F32R = mybir.dt.float32r
EXP = mybir.ActivationFunctionType.Exp
COPY = mybir.ActivationFunctionType.Copy

B, T, C, H, D = 2, 2048, 1024, 16, 64
HC = 4            # heads per core
DC = HC * D       # head-dim columns per core (256)
VC = HC * 65      # v_aug columns (per head: 64 v cols + ones col)
NT = T // 128     # 16 s/t tiles
NB = T // 512     # 4 q-blocks
KO = C // 128     # 8 contraction subtiles

_cached = {}


def split_sync_waits(nc, maxw=1):
    """This walrus build rejects instructions with >1 sync-wait; move the
    excess onto dedicated NOPs inserted just before, on the same engine."""
    n = 0
    for fn in nc.m.functions:
        for bb in fn.blocks:
            insts = bb.instructions
            i = 0
            while i < len(insts):
                inst = insts[i]
                si = getattr(inst, 'sync_info', None)
                if si is not None and si.on_wait and len(si.on_wait) > maxw:
                    waits = list(si.on_wait)
                    extra = waits[:-maxw]
                    while len(si.on_wait) > maxw:
                        si.on_wait.pop(0)
                    nops = []
                    for w in extra:
                        nop = mybir.InstNoOp(
                            name=f"I-{nc.next_id()}",
                            engine=inst.engine,
                            bass_nofuse=True,
                            sync_info=mybir.SyncInfo(on_wait=[w], on_update=[]),
                        )
                        nc.register_instruction(nop)
                        nops.append(nop)
                    insts[i:i] = nops
                    i += len(nops)
                    n += 1
                i += 1
    return n


# Attention runs in two t-phases: phase 0 covers t in [0, 1024) (s-tiles
# j<8), phase 1 covers t in [1024, 2048) (all 16 s-tiles). Each phase
# stores, per s-tile j, the causal span [max(128j, TLO), THI) contiguously.
PH_TLO = (0, 1024)
PH_THI = (1024, 2048)
PH_JS = (8, 16)
PH_OFF = []
PH_COLS = []
for _p in range(2):
    offs = []
    off = 0
    for _j in range(PH_JS[_p]):
        offs.append(off)
        off += PH_THI[_p] - max(128 * _j, PH_TLO[_p])
    PH_OFF.append(offs)
    PH_COLS.append(off)  # 4608 / 12800


def build_program(has_bias=True):
    nc = bass.Bass()

    xT_d = nc.dram_tensor("xT", [C, T], F32R, kind="ExternalInput")
    cT_d = nc.dram_tensor("cT", [C, T], F32R, kind="ExternalInput")
    wq_d = nc.dram_tensor("wq", [C, DC], F32R, kind="ExternalInput")
    wk_d = nc.dram_tensor("wk", [C, DC], F32R, kind="ExternalInput")
    wv_d = nc.dram_tensor("wv", [C, VC], F32R, kind="ExternalInput")
    wp_d = nc.dram_tensor("wp", [DC, C], F32R, kind="ExternalInput")
    bq_d = nc.dram_tensor("bq", [1, DC], F32R, kind="ExternalInput")
    bk_d = nc.dram_tensor("bk", [1, DC], F32R, kind="ExternalInput")
    bv_d = nc.dram_tensor("bv", [1, VC], F32R, kind="ExternalInput")
    ones_d = nc.dram_tensor("onesr", [1, 512], F32R, kind="ExternalInput")
    msk_d = nc.dram_tensor("msk", [128, 128], F32R, kind="ExternalInput")
    out_d = nc.dram_tensor("out", [T, C], F32, kind="ExternalOutput")

    SCALE = 1.0 / float(np.sqrt(D))
    xT_r = xT_d.rearrange("(ko p) t -> p ko t", p=128)
    cT_r = cT_d.rearrange("(ko p) t -> p ko t", p=128)

    with TileContext(nc) as tc:
        with (
            tc.tile_pool(name="const", bufs=1) as constp,
            tc.tile_pool(name="w", bufs=1) as wpool,
            tc.tile_pool(name="act", bufs=3) as actp,
            tc.tile_pool(name="qkv", bufs=1) as qkvp,

            tc.tile_pool(name="y", bufs=1) as yp,
            tc.tile_pool(name="nrm", bufs=2) as nrmp,
            tc.tile_pool(name="psQK", bufs=2, space="PSUM") as psQK,
            tc.tile_pool(name="psAV", bufs=2, space="PSUM") as psAV,
            tc.tile_pool(name="psP", bufs=2, space="PSUM") as psP,
        ):
            # ---- constants / first-needed weights ----
            ones = constp.tile([1, 512], F32R, tag="ones")
            nc.sync.dma_start(ones[:], ones_d[:])
            bk = constp.tile([1, DC], F32R, tag="bk")
            bv = constp.tile([1, VC], F32R, tag="bv")
            if has_bias:
                nc.sync.dma_start(bk[:], bk_d[:])
            wk = wpool.tile([128, KO, DC], F32R, tag="wk")
            wv = wpool.tile([128, KO, VC], F32R, tag="wv")
            wk_r = wk_d.rearrange("(ko p) d -> p ko d", p=128)
            nc.sync.dma_start(wk[:, :, 0:128], wk_r[:, :, 0:128])
            # loaded later, between projection passes (hides under compute)
            msk = constp.tile([128, 128], F32R, tag="msk")
            bq = constp.tile([1, DC], F32R, tag="bq")
            wq = wpool.tile([128, KO, DC], F32R, tag="wq")
            wp = wpool.tile([128, 2, C], F32R, tag="wp")

            # ---- persistent activations ----
            qT = qkvp.tile([128, 2, T], F32R, tag="qT")          # [dc, m, t]
            kT = qkvp.tile([128, 2, T], F32R, tag="kT")
            vA = qkvp.tile([128, NT, HC * 65], F32R, tag="vA")   # v_aug
            yT = yp.tile([128, 2, T], F32R, tag="yT")



            def load_chunk(src_r, tc4):
                # one 512-wide t-chunk of the [c, t] source, as 2 half-ko tiles
                aA = actp.tile([128, 4, 512], F32R, tag="a_in")
                aB = actp.tile([128, 4, 512], F32R, tag="a_in")
                cs = slice(512 * tc4, 512 * (tc4 + 1))
                nc.sync.dma_start(aA[:], src_r[:, 0:4, cs])
                nc.sync.dma_start(aB[:], src_r[:, 4:8, cs])

                def kslice(k, sub=None):
                    t_ = aA if k < 4 else aB
                    if sub is None:
                        return t_[:, k % 4, :]
                    return t_[:, k % 4, 128 * sub: 128 * (sub + 1)]
                return kslice

            def qk_chunk(kslice, w_t, b_t, dst, tc4):
                # dst[:, m, chunk] [128 dc, 512 t] = (w^T x)^T + bias
                for m in range(2):
                    ps = psP.tile([128, 512], F32, tag="psP")
                    for k in range(KO):
                        nc.tensor.matmul(
                            ps[:], w_t[:, k, 128 * m: 128 * (m + 1)],
                            kslice(k), start=(k == 0),
                            stop=(not has_bias and k == KO - 1))
                    if has_bias:
                        nc.tensor.matmul(
                            ps[:], b_t[:, 128 * m: 128 * (m + 1)],
                            ones[:], start=False, stop=True)
                    nc.vector.tensor_copy(
                        dst[:, m, 512 * tc4: 512 * (tc4 + 1)], ps[:])

            def v_chunk(kslice, tc4):
                for sub in range(4):
                    j = 4 * tc4 + sub
                    ps = psP.tile([128, 512], F32, tag="psP")
                    for k in range(KO):
                        nc.tensor.matmul(
                            ps[:, :VC], kslice(k, sub),
                            wv[:, k, :], start=(k == 0), stop=False)
                    # always: bv_aug also carries the v_aug ones column
                    # (softmax denominators), load-bearing even for b=0
                    nc.tensor.matmul(
                        ps[:, :VC], ones[:, :128], bv[:],
                        start=False, stop=True)
                    nc.vector.tensor_copy(vA[:, j, :], ps[:, :VC])

            def attn_phase(h, expT, ph, after_qb=None, js=None, qbs=None):
                m, lo = h // 2, 64 * (h % 2)
                kT_h = kT[lo:lo + 64, m, :]
                qT_h = qT[lo:lo + 64, m, :]
                vA_h_base = 65 * h
                TLO, THI = PH_TLO[ph], PH_THI[ph]
                OFF = PH_OFF[ph]

                for j in (js if js is not None else range(PH_JS[ph])):
                    s0 = max(128 * j, TLO)
                    slen = THI - s0
                    ps = psQK.tile([128, 1024], F32, tag="psQK")
                    for c0 in range(0, slen, 512):
                        n = min(512, slen - c0)
                        nc.tensor.matmul(
                            ps[:, c0: c0 + n],
                            kT_h[:, 128 * j: 128 * (j + 1)],
                            qT_h[:, s0 + c0: s0 + c0 + n],
                            start=True, stop=True)
                    nc.scalar.activation(
                        expT[:, OFF[j]: OFF[j] + slen],
                        ps[:, :slen], EXP, scale=SCALE)
                    if s0 == 128 * j:
                        # diagonal 128x128 at span start: keep s <= t
                        nc.vector.tensor_mul(
                            expT[:, OFF[j]: OFF[j] + 128],
                            expT[:, OFF[j]: OFF[j] + 128], msk[:])

                for qb in (qbs if qbs is not None else (2 * ph, 2 * ph + 1)):
                    ps = psAV.tile([128, 512], F32, tag="psAV")
                    njs = 4 * qb + 4
                    for j in range(njs):
                        lhsT = vA[:, j, vA_h_base: vA_h_base + 65]
                        st = (j == 0)
                        sp = (j == njs - 1)
                        if j // 4 == qb:
                            lead = 128 * j - 512 * qb
                            nc.tensor.matmul(
                                ps[:65, lead:], lhsT,
                                expT[:, OFF[j]: OFF[j] + 512 - lead],
                                start=st, stop=sp)
                        else:
                            co = OFF[j] + 512 * qb - max(128 * j, TLO)
                            nc.tensor.matmul(
                                ps[:65, :], lhsT, expT[:, co: co + 512],
                                start=st, stop=sp)
                    rcp = nrmp.tile([1, 512], F32R, tag="rcp")
                    with nc.allow_low_precision("f32r softmax denominators"):
                        nc.vector.reciprocal(rcp[:], ps[64:65, :])
                    # broadcast 1/denom across 64 partitions via K=1 matmul
                    rbc = psP.tile([128, 512], F32, tag="psP")
                    nc.tensor.matmul(rbc[:64, :], ones[:, :64], rcp[:],
                                     start=True, stop=True)
                    rbs = nrmp.tile([64, 512], F32, tag="rbs")
                    nc.vector.tensor_copy(rbs[:], rbc[:64, :])
                    nc.vector.tensor_mul(
                        yT[lo:lo + 64, m, 512 * qb: 512 * (qb + 1)],
                        ps[:64, :], rbs[:])
                    if after_qb is not None and qb in after_qb:
                        after_qb[qb]()

            def out_proj_tile(tt):
                ts = slice(128 * tt, 128 * (tt + 1))
                for co in range(2):
                    ps = psP.tile([128, 512], F32, tag="psP")
                    for m in range(2):
                        nc.tensor.matmul(
                            ps[:], yT[:, m, ts],
                            wp[:, m, 512 * co: 512 * (co + 1)],
                            start=(m == 0), stop=(m == 1))
                    ob = obp[0].tile([128, 512], F32, tag="ob")
                    if (tt + co) % 2:
                        nc.vector.tensor_copy(ob[:], ps[:])
                    else:
                        nc.scalar.activation(ob[:], ps[:], COPY)
                    nc.sync.dma_start(out_d[ts, 512 * co: 512 * (co + 1)], ob[:])

            # ---- emission order (pipelines ACT exp against PE) ----
            def proj_pair(tc4, first=False):
                ksc = load_chunk(cT_r, tc4)
                if first:
                    nc.sync.dma_start(wk[:, :, 128:256], wk_r[:, :, 128:256])
                    nc.sync.dma_start(
                        wv[:], wv_d.rearrange("(ko p) d -> p ko d", p=128))
                    nc.sync.dma_start(bv[:], bv_d[:])
                ksx = load_chunk(xT_r, tc4)
                if first:
                    nc.sync.dma_start(
                        wq[:], wq_d.rearrange("(ko p) d -> p ko d", p=128))
                    if has_bias:
                        nc.sync.dma_start(bq[:], bq_d[:])
                    nc.sync.dma_start(msk[:], msk_d[:])
                qk_chunk(ksc, wk, bk, kT, tc4)
                v_chunk(ksc, tc4)
                qk_chunk(ksx, wq, bq, qT, tc4)

            obp = [None]
            with tc.tile_pool(name="expt2", bufs=1) as exptp2:
                expT2 = exptp2.tile([128, PH_COLS[1]], F32R, tag="expT2")
                with tc.tile_pool(name="expt1", bufs=1) as exptp1:
                    expT1 = exptp1.tile([128, PH_COLS[0]], F32R, tag="expT1")
                    proj_pair(0, first=True)
                    proj_pair(1)
                    attn_phase(0, expT1, 0)
                    proj_pair(2)
                    attn_phase(1, expT1, 0)
                    nc.sync.dma_start(
                        wp[:], wp_d.rearrange("(m p) c -> p m c", p=128))
                    attn_phase(2, expT1, 0)
                    proj_pair(3)
                    attn_phase(3, expT1, 0)

                with tc.tile_pool(name="ob", bufs=8) as _obp:
                    obp[0] = _obp
                    attn_phase(0, expT2, 1)
                    for tt in range(0, 4):
                        out_proj_tile(tt)
                    attn_phase(1, expT2, 1)
                    for tt in range(4, 8):
                        out_proj_tile(tt)
                    attn_phase(2, expT2, 1)
                    attn_phase(3, expT2, 1, after_qb={
                        2: lambda: [out_proj_tile(tt) for tt in range(8, 12)],
                        3: lambda: [out_proj_tile(tt) for tt in range(12, 16)],
                    })

    split_sync_waits(nc)
    return nc


def kernel(x, context, Wq, bq, Wkv, bkv, Wp, bp):
    from concourse.bass_utils import run_bass_kernel_spmd

    x = np.asarray(x, dtype=np.float32)
    context = np.asarray(context, dtype=np.float32)
    Wq = np.asarray(Wq, dtype=np.float32)
    Wkv = np.asarray(Wkv, dtype=np.float32)
    Wp_a = np.asarray(Wp, dtype=np.float32)
    bq_a = np.asarray(bq, dtype=np.float32)
    bkv_a = np.asarray(bkv, dtype=np.float32)
    bp_a = np.asarray(bp, dtype=np.float32)

    has_bias = bool(np.any(bq_a) or np.any(bkv_a))
    key = ('nc', has_bias)
    if key not in _cached:
        _cached[key] = build_program(has_bias)
    nc = _cached[key]

    msk = np.triu(np.ones((128, 128), dtype=np.float32))
    xT = [np.ascontiguousarray(x[b].T) for b in range(B)]
    cT = [np.ascontiguousarray(context[b].T) for b in range(B)]

    ones512 = np.ones((1, 512), dtype=np.float32)
    in_maps = []
    for c in range(8):
        b, hg = c // 4, c % 4
        hs = slice(DC * hg, DC * (hg + 1))
        vs = slice(C + DC * hg, C + DC * (hg + 1))
        wv_aug = np.zeros((C, VC), dtype=np.float32)
        bv_aug = np.zeros((1, VC), dtype=np.float32)
        for h in range(HC):
            wv_aug[:, 65 * h: 65 * h + 64] = Wkv[:, C + DC * hg + 64 * h:
                                                 C + DC * hg + 64 * (h + 1)]
            bv_aug[0, 65 * h: 65 * h + 64] = bkv_a[C + DC * hg + 64 * h:
                                                   C + DC * hg + 64 * (h + 1)]
            bv_aug[0, 65 * h + 64] = 1.0
        in_maps.append({
            "xT": xT[b], "cT": cT[b],
            "wq": np.ascontiguousarray(Wq[:, hs]),
            "wk": np.ascontiguousarray(Wkv[:, hs]),
            "wv": wv_aug,
            "wp": np.ascontiguousarray(Wp_a[hs, :]),
            "bq": np.ascontiguousarray(bq_a[hs]).reshape(1, DC),
            "bk": np.ascontiguousarray(bkv_a[hs]).reshape(1, DC),
            "bv": bv_aug,
            "msk": msk, "onesr": ones512,
        })

    res = run_bass_kernel_spmd(nc, in_maps, list(range(8)))
    out = np.zeros((B, T, C), dtype=np.float32)
    for c in range(8):
        out[c // 4] += res.results[c]["out"]
    out += bp_a[None, None, :]
    return out
